# revision 28
# baseline (speedup 1.0000x reference)
"""Trainium2 Bass kernel for BatchHardTripletLoss (topk_masking).

Strategy (8 NeuronCores, data-parallel over anchor rows):
  - Host rotates the concatenated batch per core so every core's program is
    identical (SPMD): core c gets batch_rot[l] = B[(l + 1024*c) % 8192].
    Core c's "own" rows are local rows [0, 1024); the positive partner of
    local row l is local row 4096 + l.
  - Column-chunk pipeline: the batch is loaded in 4 chunks of 2048 rows.
    While chunk q is DMA'd in and transposed into B^T (PE), the main-loop
    matmuls for the columns of chunk q-1 run, so the input load/transpose
    almost fully overlaps compute.
  - Main loop per (strip rt, chunk q): S[i,j] = b_i.b_j - 0.5*||b_j||^2 via
    2 fp32r K=128 matmuls + 1 bf16 K=128 bias matmul (rhs rows 0-1 hold
    -0.5*colsq split into exact bf16 hi+lo).  The self/partner diagonal
    mask is applied during the PSUM->SBUF copy as a fused +(-BIG diagonal)
    tensor op on the DVE; the PSUM copy of the partner block doubles as the
    source for hp: its pre-mask diagonal is extracted with a fused
    multiply-by-identity + row-sum, giving hp = rowsq_i - 2*diag.
  - Top-8 of S per row via DVE max8 per (strip, chunk), merged per strip;
    hn = rowsq_i - 2*S_k is the exact (k_sel+1)-th smallest masked
    distance.  Softplus triplet-loss partials are reduced to 5 scalars.
  - Host reduces the 8 cores' partials into the 5 reference outputs.

Perf notes (CoreSim p-state model): the PE ramps 0.65 -> 1.2 -> 2.4 GHz
only while executing without BLOCKING semaphore waits; pre-satisfied waits
are free.  All PE matmuls are K=128 (K=2 matmuls never ramp past 1.2 GHz)
and consumers (copies on ACT, masked copies/max8 on DVE) are kept faster
than the PE's full-clock pace so its waits stay pre-satisfied.
"""

import numpy as np

M = 8192          # 2N total rows
D = 256           # feature dim
NCORES = 8
RPC = M // NCORES  # rows per core (1024)
NSTR = RPC // 128  # strips per core (8)
TN = 512           # matmul free-dim tile
NCT = M // TN      # col tiles (16)
NCH = 4            # row/col chunks (2048 each)
CPC = NCT // NCH   # col tiles per chunk (4)
BIG = 1e30
BETA = 3.0
EPS_REL = 1e-5

_cache = {}


def _build(k_sel: int):
    import concourse.bacc as bacc
    import concourse.bass as bass
    import concourse.mybir as mybir
    import concourse.tile as tile
    from contextlib import ExitStack
    from concourse.masks import make_identity

    f32 = mybir.dt.float32
    f32r = mybir.dt.float32r
    bf16 = mybir.dt.bfloat16
    AF = mybir.ActivationFunctionType
    OP = mybir.AluOpType
    AX = mybir.AxisListType

    nc = bacc.Bacc("TRN2", target_bir_lowering=False, debug=False,
                   num_devices=NCORES)
    batch = nc.dram_tensor("batch", [M, D], f32, kind="ExternalInput")
    out_d = nc.dram_tensor("out", [8], f32, kind="ExternalOutput")
    colsq_dram = nc.dram_tensor("colsq_scratch", [2, M], bf16)

    with tile.TileContext(nc) as tc, ExitStack() as ctx:
        consts = ctx.enter_context(tc.tile_pool(name="consts", bufs=1))

        ident_f = consts.tile([128, 128], f32)
        make_identity(nc, ident_f[:])

        rsq_all = consts.tile([128, 64], f32)   # ||b_l||^2, tile-major
        pdot = consts.tile([128, 8], f32)       # diag of partner S block
        top8 = consts.tile([128, 64], f32)      # per-strip top-8 of S
        cands = consts.tile([128, NSTR * 8 * NCH], f32)  # per-chunk top-8s
        bt0 = consts.tile([128, M], f32r)       # B^T chunk d=[0,128)
        bt1 = consts.tile([128, M], f32r)       # B^T chunk d=[128,256)
        bts = (bt0, bt1)

        # bias rhs: rows 0-1 = -0.5*colsq split into exact bf16 hi+lo,
        # rest zero; lhsT selector = ones at rows 0-1 (K=128 so the PE
        # p-state ramps; a K=2 matmul is stuck at 1.2 GHz)
        bm = consts.tile([128, M], bf16)
        nc.gpsimd.memset(bm[:], 0.0)
        sel2 = consts.tile([128, 128], bf16)
        nc.gpsimd.memset(sel2[:], 0.0)
        nc.gpsimd.memset(sel2[0:2, :], 1.0)

        # negdiag[v]: -BIG at (p, 128*v + p), 0 elsewhere -- added during
        # the PSUM->SBUF copy of a masked tile (affine_select fills where
        # the affine condition is false: p - j + 128*v == 0 -> diagonal)
        negdiag = consts.tile([128, 4 * TN], f32)
        nc.gpsimd.memset(negdiag[:], 0.0)
        for v in range(4):
            nd = negdiag[:, TN * v:TN * (v + 1)]
            nc.gpsimd.affine_select(
                out=nd, in_=nd, compare_op=OP.not_equal, fill=-BIG,
                base=128 * v, pattern=[[-1, TN]], channel_multiplier=1)

        rm_pool = ctx.enter_context(tc.tile_pool(name="rm", bufs=3))
        mpsum = ctx.enter_context(
            tc.tile_pool(name="mpsum", bufs=8, space="PSUM"))
        strip_pool = ctx.enter_context(tc.tile_pool(name="strip", bufs=3))
        scr_pool = ctx.enter_context(tc.tile_pool(name="scr", bufs=3))
        cpool = ctx.enter_context(tc.tile_pool(name="cpool", bufs=2))

        bview = batch.ap().rearrange("(t p) d -> p t d", p=128)
        rms = {}

        def emit_dma(q):
            rm = rm_pool.tile([128, 16 * D], f32)
            rmv = rm[:].rearrange("p (t d) -> p t d", d=D)
            nc.sync.dma_start(rmv[:, 0:8, :],
                              bview[:, 16 * q:16 * q + 8, :])
            nc.sync.dma_start(rmv[:, 8:16, :],
                              bview[:, 16 * q + 8:16 * q + 16, :])
            rms[q] = rm

        def emit_squares(q, tt0, tt1):
            rm = rms[q]
            for tt in range(tt0, tt1):
                t = 16 * q + tt
                rmt = rm[:, D * tt:D * (tt + 1)]
                if tt < 10:
                    scr = scr_pool.tile([128, D], f32, tag="scr")
                    nc.vector.scalar_tensor_tensor(
                        out=scr[:], in0=rmt, scalar=1.0, in1=rmt,
                        op0=OP.mult, op1=OP.mult,
                        accum_out=rsq_all[:, t:t + 1])
                else:
                    scr = scr_pool.tile([128, D], f32, tag="scr")
                    nc.scalar.activation(
                        scr[:], rmt, AF.Square,
                        accum_out=rsq_all[:, t:t + 1])

        def emit_tg(q, u):
            # transpose group u: row-tiles 2u, 2u+1 of chunk q, both
            # 128-col halves -> one 4-slice psum tile, strided copy-out
            rm = rms[q]
            pt = mpsum.tile([128, TN], f32, tag="ps")
            for j in range(2):
                tt = 2 * u + j
                for kc in range(2):
                    src = rm[:, D * tt + 128 * kc:D * tt + 128 * kc + 128]
                    nc.tensor.transpose(
                        pt[:, 128 * (2 * j + kc):128 * (2 * j + kc + 1)],
                        src, ident_f[:])
            base = 128 * (16 * q + 2 * u)
            psv = pt[:].rearrange("p (a b) -> p a b", b=128)
            for kc in range(2):
                dst = bts[kc][:, base:base + 256]
                dstv = dst.rearrange("p (a b) -> p a b", b=128)
                if u % 2 == 0:
                    nc.vector.tensor_copy(dstv, psv[:, kc:4:2, :])
                else:
                    nc.scalar.activation(dstv, psv[:, kc:4:2, :], AF.Copy)

        def emit_colsq_part(q):
            # transpose rsq_all[:, 16q:16q+16] -> [16,128], scale -0.5,
            # exact bf16 hi+lo split, DRAM round-trip into bm rows 0-1
            pt = mpsum.tile([128, TN], f32, tag="ps")
            nc.tensor.transpose(pt[0:16, 0:128],
                                rsq_all[:, 16 * q:16 * q + 16], ident_f[:])
            sqTf = cpool.tile([16, 128], f32, tag="sqTf")
            nc.scalar.activation(sqTf[:], pt[0:16, 0:128], AF.Copy,
                                 scale=-0.5)
            chi = cpool.tile([16, 128], bf16, tag="chi")
            nc.scalar.activation(chi[:], sqTf[:], AF.Copy)
            chif = cpool.tile([16, 128], f32, tag="chif")
            nc.vector.tensor_copy(chif[:], chi[:])
            clo = cpool.tile([16, 128], bf16, tag="clo")
            nc.vector.tensor_sub(clo[:], sqTf[:], chif[:])
            dv0 = colsq_dram.ap()[0, 2048 * q:2048 * (q + 1)]
            dv1 = colsq_dram.ap()[1, 2048 * q:2048 * (q + 1)]
            nc.sync.dma_start(dv0.rearrange("(t p) -> t p", p=128), chi[:])
            nc.sync.dma_start(dv1.rearrange("(t p) -> t p", p=128), clo[:])
            nc.sync.dma_start(bm[0:2, 2048 * q:2048 * (q + 1)],
                              colsq_dram.ap()[:, 2048 * q:2048 * (q + 1)])

        def emit_main_unit(rt, q):
            # strip rt x the 4 column tiles of chunk q
            stri = strip_pool.tile([128, CPC * TN], f32)
            for cc in range(CPC):
                ct = CPC * q + cc
                ps = mpsum.tile([128, TN], f32, tag="ps")
                self_m = ct == rt // 4
                part_m = ct == 8 + rt // 4
                nc.tensor.matmul(
                    ps[:],
                    lhsT=bt0[:, 128 * rt:128 * rt + 128],
                    rhs=bt0[:, TN * ct:TN * (ct + 1)],
                    start=True, stop=False)
                nc.tensor.matmul(
                    ps[:],
                    lhsT=bt1[:, 128 * rt:128 * rt + 128],
                    rhs=bt1[:, TN * ct:TN * (ct + 1)],
                    start=False, stop=False)
                nc.tensor.matmul(
                    ps[:], lhsT=sel2[:],
                    rhs=bm[:, TN * ct:TN * (ct + 1)],
                    start=False, stop=True)
                dst = stri[:, TN * cc:TN * (cc + 1)]
                if self_m or part_m:
                    off = 128 * (rt % 4)
                    if part_m:
                        # hp source: pre-mask diagonal of the partner block
                        scr = scr_pool.tile([128, 128], f32, tag="dscr")
                        nc.vector.scalar_tensor_tensor(
                            out=scr[:], in0=ps[:, off:off + 128],
                            scalar=1.0, in1=ident_f[:],
                            op0=OP.mult, op1=OP.mult,
                            accum_out=pdot[:, rt:rt + 1])
                    # masked copy: stri = ps + (-BIG on the diagonal)
                    nc.vector.scalar_tensor_tensor(
                        out=dst, in0=ps[:], scalar=1.0,
                        in1=negdiag[:, TN * (rt % 4):TN * (rt % 4 + 1)],
                        op0=OP.mult, op1=OP.add)
                else:
                    nc.scalar.activation(dst, ps[:], AF.Copy)
            nc.vector.max(out=cands[:, 32 * rt + 8 * q:32 * rt + 8 * q + 8],
                          in_=stri[:])

        # ---------------- pipeline ----------------
        emit_dma(0)
        emit_dma(1)
        for u in range(8):
            emit_tg(0, u)
            emit_squares(0, 2 * u, 2 * u + 2)
        emit_colsq_part(0)

        for q in range(1, NCH):
            if q + 1 < NCH:
                emit_dma(q + 1)
            for u in range(8):
                emit_tg(q, u)
                emit_squares(q, 2 * u, 2 * u + 2)
            for rt in range(NSTR):
                emit_main_unit(rt, q - 1)
            emit_colsq_part(q)

        for rt in range(NSTR):
            emit_main_unit(rt, NCH - 1)
            nc.vector.max(out=top8[:, 8 * rt:8 * rt + 8],
                          in_=cands[:, 32 * rt:32 * rt + 32])

        # ---------------- finalize: hp/hn, softplus, partial sums --------
        fin = ctx.enter_context(tc.tile_pool(name="fin", bufs=1))

        _ftn = [0]

        def ft():
            _ftn[0] += 1
            return fin.tile([128, 8], f32, tag="fin8", bufs=4,
                            name=f"fin8_{_ftn[0]}")

        rsq_own = rsq_all[:, 0:8]
        tk = top8[:, k_sel:64:8]

        hn = fin.tile([128, 8], f32)
        nc.vector.tensor_scalar(hn[:], tk, -2.0, None, op0=OP.mult)
        nc.vector.tensor_add(hn[:], hn[:], rsq_own)
        # hp = rowsq_i - 2 * premask_diag(partner block)
        hp = fin.tile([128, 8], f32)
        nc.vector.tensor_scalar(hp[:], pdot[:], -2.0, None, op0=OP.mult)
        nc.vector.tensor_add(hp[:], hp[:], rsq_own)
        diff = fin.tile([128, 8], f32)
        nc.vector.tensor_sub(diff[:], hp[:], hn[:])

        # softplus(3*diff) = relu(3d) + log1p(exp(-|3d|))
        ax = ft()
        nc.scalar.activation(ax[:], diff[:], AF.Abs, scale=BETA)
        en = ft()
        nc.scalar.activation(en[:], ax[:], AF.Exp, scale=-1.0)
        ln1 = ft()
        nc.scalar.activation(ln1[:], en[:], AF.Ln, bias=1.0)
        rl = ft()
        nc.scalar.activation(rl[:], diff[:], AF.Relu, scale=BETA)
        sp = fin.tile([128, 8], f32)
        nc.vector.tensor_add(sp[:], ln1[:], rl[:])

        p5 = fin.tile([128, 8], f32)
        nc.vector.memset(p5[:], 0.0)
        relm = ft()
        nc.vector.tensor_scalar(relm[:], sp[:], float(EPS_REL * BETA), None,
                                op0=OP.is_gt, op1=OP.add,
                                accum_out=p5[:, 1:2])
        tlrel = ft()
        nc.vector.tensor_mul(tlrel[:], sp[:], relm[:])
        nc.vector.reduce_sum(p5[:, 0:1], tlrel[:], axis=AX.X)
        nc.vector.reduce_sum(p5[:, 2:3], diff[:], axis=AX.X)
        goodm = ft()
        nc.vector.tensor_scalar(goodm[:], diff[:], 0.0, None,
                                op0=OP.is_lt, op1=OP.add,
                                accum_out=p5[:, 3:4])
        nc.vector.reduce_sum(p5[:, 4:5], rsq_own, axis=AX.X)

        pf = mpsum.tile([128, TN], f32, tag="ps")
        nc.tensor.transpose(pf[0:8, 0:128], p5[:], ident_f[:])
        p5T = fin.tile([8, 128], f32)
        nc.vector.tensor_copy(p5T[:], pf[0:8, 0:128])
        o8 = fin.tile([8, 1], f32)
        nc.vector.reduce_sum(o8[:], p5T[:], axis=AX.X)
        nc.sync.dma_start(out_d.ap(), o8[:])

    nc.compile()
    return nc


def _get_program(k_sel: int):
    if k_sel not in _cache:
        _cache[k_sel] = _build(k_sel)
    return _cache[k_sel]


def run_sharded(B: np.ndarray, k_sel: int, trace: bool = False):
    """Run the SPMD kernel on 8 cores. Returns (partials [8,8], exec_time_ns)."""
    from concourse.bass_utils import run_bass_kernel_spmd

    nc = _get_program(k_sel)
    in_maps = [{"batch": np.ascontiguousarray(np.roll(B, -RPC * c, axis=0))}
               for c in range(NCORES)]
    res = run_bass_kernel_spmd(nc, in_maps, core_ids=list(range(NCORES)),
                               trace=trace)
    parts = np.stack([res.results[c]["out"] for c in range(NCORES)])
    return parts, res.exec_time_ns


def _combine(parts: np.ndarray):
    s = parts.astype(np.float64).sum(axis=0)
    sum_tl = s[0] / BETA
    cnt = s[1]
    mean_relevant = np.float32(sum_tl / cnt)
    mean_diff = np.float32(s[2] / M)
    good = np.int32(int(round(s[3])))
    bad = np.int32(M - int(good))
    mean_norm = np.float32(np.sqrt(s[4] / M))
    return (mean_relevant, mean_diff, good, bad, mean_norm)


def kernel(h1: np.ndarray, h2: np.ndarray, k_sel=3):
    k = int(np.asarray(k_sel))
    assert 0 <= k <= 7, f"k_sel={k} out of supported range"
    B = np.concatenate([np.asarray(h1, dtype=np.float32),
                        np.asarray(h2, dtype=np.float32)], axis=0)
    assert B.shape == (M, D)
    parts, _ = run_sharded(B, k)
    return _combine(parts)
